# revision 7
# baseline (speedup 1.0000x reference)
"""KAN layer (B=8192, IN_F=OUT_F=1024, GRID=5) on 8 Trainium2 cores.

Math: Y[b,o] = W0[o]*silu(x) + spline_o(clip(x,-1,1)) + b[o], x = X[b,o].
The degree-1 B-spline is evaluated in the *segment* basis
    spline(clip(x)) = A''[o] + sum_j gamma_j[o] * v_j(x),
    v_j(x) = clip(x, s_{j-1}, s_j),  knots s = (-1,-0.5,0,0.5,1),
    gamma_j = w1 * m_j (segment slopes),
so each map is a 2-op tensor_scalar clip straight from x.

Sharding: edges across the 8 cores (128 edges/core, full batch 8192 on the
free dim).  Per core TensorE does a per-edge diagonal combine of 5 feature
maps into PSUM: v2,v3 ride ONE fp8e4 DoubleRow matmul (2 maps/pass), v1,v4
and silu are fp16 matmuls.  ScalarE: silu + most of the PSUM evacuation
(Identity+bias); VectorE: the 4 clips + the evac remainder.  I/O is fp16
(host converts); fp8 weight-quantization error is minimax-compensated into
the per-edge bias on host.
"""
import sys

for _p in ("/root/.axon_site", "/root/.axon_site/_ro/trn_rl_repo", "/root/.axon_site/_ro/pypackages"):
    if _p not in sys.path:
        sys.path.append(_p)

import numpy as np
import ml_dtypes

import concourse.bacc as bacc
import concourse.tile as tile
from concourse import mybir
from concourse.bass_utils import run_bass_kernel_spmd

B, IN_F, OUT_F, GRID = 8192, 1024, 1024, 5
N_CORES = 8
PER = OUT_F // N_CORES          # 128 edges per core
NB = B                          # 8192 batch columns per core
SBLK = 2048                     # superblock columns
NSB = NB // SBLK                # 4 superblocks
CHUNK = 512                     # one PSUM bank of fp32
SPLIT = 1792                    # evac columns on ScalarE per superblock
SPLIT_H = 896                   # evac columns on ScalarE per half-superblock

_nc_cache = None


def _build():
    f32 = mybir.dt.float32
    f16 = mybir.dt.float16
    f8 = mybir.dt.float8e4
    AF = mybir.ActivationFunctionType
    OP = mybir.AluOpType
    DRm = mybir.MatmulPerfMode.DoubleRow

    nc = bacc.Bacc("TRN2", target_bir_lowering=False, debug=False)
    xt = nc.dram_tensor("xt", [PER, NB], f16, kind="ExternalInput").ap()
    cpack = nc.dram_tensor("cpack", [PER, 8], f32, kind="ExternalInput").ap()
    ident = nc.dram_tensor("ident", [PER, 128], f16, kind="ExternalInput").ap()
    yt = nc.dram_tensor("yt", [PER, NB], f16, kind="ExternalOutput").ap()

    with tile.TileContext(nc) as tc:
        with tc.tile_pool(name="const", bufs=1) as cpool, \
             tc.tile_pool(name="xin", bufs=1) as xpool, \
             tc.tile_pool(name="sil", bufs=1) as spool, \
             tc.tile_pool(name="v14", bufs=2) as vpool, \
             tc.tile_pool(name="v23", bufs=2) as wpool, \
             tc.tile_pool(name="yout", bufs=2) as ypool, \
             tc.tile_pool(name="ps", bufs=2, space="PSUM") as pspool:
            # consts via the scalar-HWDGE ring (fast, ScalarE idle early)
            cp = cpool.tile([128, 8], f32)
            nc.scalar.dma_start(cp[:], cpack[:, :])
            id16 = cpool.tile([128, 128], f16)
            nc.scalar.dma_start(id16[:], ident[:, :])

            # input loads on the sync/HWDGE ring; small first chunk
            x0 = xpool.tile([128, SBLK], f16, tag="x0", name="x0")
            nc.sync.dma_start(x0[:, 0:1024], xt[:, 0:1024])
            nc.sync.dma_start(x0[:, 1024:SBLK], xt[:, 1024:SBLK])
            x1 = xpool.tile([128, SBLK], f16, tag="x1", name="x1")
            nc.sync.dma_start(x1[:], xt[:, SBLK:2 * SBLK])
            x23 = xpool.tile([128, 2 * SBLK], f16, tag="x23", name="x23")
            nc.sync.dma_start(x23[:], xt[:, 2 * SBLK:4 * SBLK])

            scr = cpool.tile([128, CHUNK], f16)
            nc.vector.memset(scr[:], 0.25)
            # silu ACT-table load overlaps the first input DMA
            dum = cpool.tile([128, 1], f16)
            nc.scalar.activation(dum[:], scr[:, 0:1], AF.Silu)

            # PE warm-up so HAM reaches 8/8 right as real matmuls arrive
            pswarm = pspool.tile([128, SBLK], f32, tag="ps", name="pswarm")
            for r in range(12):
                nc.tensor.matmul(pswarm[:, 0:CHUNK], scr[:, 0:128], scr[:],
                                 start=True, stop=True, skip_group_check=True)

            # per-edge diagonal stationaries (on-device from ident * weight)
            dsil = cpool.tile([128, 128], f16)
            nc.vector.tensor_scalar_mul(dsil[:], id16[:], cp[:, 0:1])
            dv1 = cpool.tile([128, 128], f16)
            nc.vector.tensor_scalar_mul(dv1[:], id16[:], cp[:, 1:2])
            dp23 = cpool.tile([128, 2, 128], f8)
            nc.vector.tensor_scalar_mul(dp23[:, 0, :], id16[:], cp[:, 2:3])
            nc.vector.tensor_scalar_mul(dp23[:, 1, :], id16[:], cp[:, 3:4])
            dv4 = cpool.tile([128, 128], f16)
            nc.vector.tensor_scalar_mul(dv4[:], id16[:], cp[:, 4:5])

            def clips(v23, v1, v4, xv, cl):
                nc.vector.tensor_scalar(v23[:, 0, cl], xv[:, cl], 0.0, -0.5,
                                        OP.min, OP.max)
                nc.vector.tensor_scalar(v23[:, 1, cl], xv[:, cl], 0.5, 0.0,
                                        OP.min, OP.max)
                nc.vector.tensor_scalar(v1[:, cl], xv[:, cl], -0.5, -1.0,
                                        OP.min, OP.max)
                nc.vector.tensor_scalar(v4[:, cl], xv[:, cl], 1.0, 0.5,
                                        OP.min, OP.max)

            def mm_group(ps, v23, v1, v4, sil_ap, lo, hi, off):
                """Chunks [lo,hi) of ps; feature APs indexed from chunk off."""
                for c in range(lo, hi):
                    f = c - off
                    nc.tensor.matmul(ps[:, c * CHUNK:(c + 1) * CHUNK],
                                     dp23[:, 0:2, :],
                                     v23[:, 0:2, f * CHUNK:(f + 1) * CHUNK],
                                     start=True, stop=False, perf_mode=DRm,
                                     skip_group_check=True)
                for c in range(lo, hi):
                    f = c - off
                    nc.tensor.matmul(ps[:, c * CHUNK:(c + 1) * CHUNK], dv1[:],
                                     v1[:, f * CHUNK:(f + 1) * CHUNK],
                                     start=False, stop=False, skip_group_check=True)
                for c in range(lo, hi):
                    f = c - off
                    nc.tensor.matmul(ps[:, c * CHUNK:(c + 1) * CHUNK], dv4[:],
                                     v4[:, f * CHUNK:(f + 1) * CHUNK],
                                     start=False, stop=False, skip_group_check=True)
                for c in range(lo, hi):
                    f = c - off
                    nc.tensor.matmul(ps[:, c * CHUNK:(c + 1) * CHUNK], dsil[:],
                                     sil_ap[:, f * CHUNK:(f + 1) * CHUNK],
                                     start=False, stop=True, skip_group_check=True)

            def evac(y, ps, a, b, sp):
                nc.scalar.activation(y[:, a:sp], ps[:, a:sp], AF.Identity,
                                     bias=cp[:, 5:6], scale=1.0)
                nc.vector.tensor_scalar(y[:, sp:b], ps[:, sp:b],
                                        cp[:, 5:6], None, OP.add)

            # ---- SB0: two halves for a fast ramp ----
            ps0 = pspool.tile([128, SBLK], f32, tag="ps", name="ps0")
            y0 = ypool.tile([128, SBLK], f16, tag="y", name="y0")
            v23_0 = wpool.tile([128, 2, SBLK], f8, tag="v23", name="v23_0")
            v1_0 = vpool.tile([128, SBLK], f16, tag="v1", name="v1_0")
            v4_0 = vpool.tile([128, SBLK], f16, tag="v4", name="v4_0")
            for h in range(2):
                cl = slice(h * 1024, (h + 1) * 1024)
                clips(v23_0, v1_0, v4_0, x0, cl)
                sl = spool.tile([128, 1024], f16, tag=f"sil0{h}", name=f"sil0{h}")
                nc.scalar.activation(sl[:], x0[:, cl], AF.Silu)
                mm_group(ps0, v23_0[:, :, cl], v1_0[:, cl], v4_0[:, cl], sl[:],
                         2 * h, 2 * h + 2, 2 * h)
            evac(y0, ps0, 0, SBLK, SPLIT)
            nc.sync.dma_start(yt[:, 0:SBLK], y0[:])

            # ---- SB1 ----
            ps1 = pspool.tile([128, SBLK], f32, tag="ps", name="ps1")
            y1 = ypool.tile([128, SBLK], f16, tag="y", name="y1")
            v23_1 = wpool.tile([128, 2, SBLK], f8, tag="v23", name="v23_1")
            v1_1 = vpool.tile([128, SBLK], f16, tag="v1", name="v1_1")
            v4_1 = vpool.tile([128, SBLK], f16, tag="v4", name="v4_1")
            sil1 = spool.tile([128, SBLK], f16, tag="sil1", name="sil1")
            nc.scalar.activation(sil1[:], x1[:], AF.Silu)
            clips(v23_1, v1_1, v4_1, x1, slice(0, SBLK))
            mm_group(ps1, v23_1, v1_1, v4_1, sil1[:], 0, 4, 0)
            evac(y1, ps1, 0, SBLK, SPLIT)
            nc.sync.dma_start(yt[:, SBLK:2 * SBLK], y1[:])

            # ---- SB2+SB3: FD=4096 features, per-SB matmul/evac ----
            sil23 = spool.tile([128, 2 * SBLK], f16, tag="sil23", name="sil23")
            nc.scalar.activation(sil23[:], x23[:], AF.Silu)
            v23_23 = wpool.tile([128, 2, 2 * SBLK], f8, tag="v23w", name="v23_23")
            v1_23 = vpool.tile([128, 2 * SBLK], f16, tag="v1w", name="v1_23")
            v4_23 = vpool.tile([128, 2 * SBLK], f16, tag="v4w", name="v4_23")
            clips(v23_23, v1_23, v4_23, x23, slice(0, 2 * SBLK))

            ps2 = pspool.tile([128, SBLK], f32, tag="ps", name="ps2")
            y2 = ypool.tile([128, SBLK], f16, tag="y", name="y2")
            mm_group(ps2, v23_23[:, :, 0:SBLK], v1_23[:, 0:SBLK],
                     v4_23[:, 0:SBLK], sil23[:, 0:SBLK], 0, 4, 0)
            evac(y2, ps2, 0, SBLK, SPLIT)
            nc.sync.dma_start(yt[:, 2 * SBLK:3 * SBLK], y2[:])

            ps3 = pspool.tile([128, SBLK], f32, tag="ps", name="ps3")
            y3 = ypool.tile([128, SBLK], f16, tag="y", name="y3")
            for h in range(2):
                cl = slice(SBLK + h * 1024, SBLK + (h + 1) * 1024)
                mm_group(ps3, v23_23[:, :, cl], v1_23[:, cl], v4_23[:, cl],
                         sil23[:, cl], 2 * h, 2 * h + 2, 2 * h)
                evac(y3, ps3, h * 1024, (h + 1) * 1024, h * 1024 + SPLIT_H)
                nc.sync.dma_start(
                    yt[:, 3 * SBLK + h * 1024:3 * SBLK + (h + 1) * 1024],
                    y3[:, h * 1024:(h + 1) * 1024])
    nc.compile()
    return nc


def _host_prep(X, coeffs, W, b):
    """Per-core cpack [128, 8] fp32: W0, g1, g2, g3, g4, A'' (compensated)."""
    c = coeffs.astype(np.float64)
    W64 = W.astype(np.float64)
    b64 = b.astype(np.float64)
    m = 2.0 * (c[:, 1:] - c[:, :-1])          # [O, 4] segment slopes
    w1 = W64[:, 1]
    gam = w1[:, None] * m                      # [O, 4]
    s = np.array([-1.0, -0.5, 0.0, 0.5])
    A = b64 + w1 * c[:, 0] - (gam * s[None, :]).sum(1)
    # minimax compensation of fp8e4 quantization of g2, g3 (device uses RNE)
    d2 = gam[:, 1].astype(ml_dtypes.float8_e4m3).astype(np.float64) - gam[:, 1]
    d3 = gam[:, 2].astype(ml_dtypes.float8_e4m3).astype(np.float64) - gam[:, 2]
    cand = np.stack([-0.5 * d2, np.zeros_like(d2), 0.5 * d3], 1)
    A = A - (cand.max(1) + cand.min(1)) / 2

    cpack = np.zeros((OUT_F, 8), dtype=np.float32)
    cpack[:, 0] = W64[:, 0]
    cpack[:, 1:5] = gam
    cpack[:, 5] = A
    return cpack


def kernel(X, coeffs, W, b):
    global _nc_cache
    if _nc_cache is None:
        _nc_cache = _build()
    nc = _nc_cache

    cpack = _host_prep(X, coeffs, W, b)
    ident = np.eye(128, dtype=np.float16)
    X16 = X.astype(np.float16)
    in_maps = []
    for cidx in range(N_CORES):
        sl = slice(cidx * PER, (cidx + 1) * PER)
        in_maps.append({
            "xt": np.ascontiguousarray(X16[:, sl].T),
            "cpack": np.ascontiguousarray(cpack[sl]),
            "ident": ident,
        })

    res = run_bass_kernel_spmd(nc, in_maps, core_ids=list(range(N_CORES)))
    Y = np.empty((B, OUT_F), dtype=np.float32)
    for cidx in range(N_CORES):
        sl = slice(cidx * PER, (cidx + 1) * PER)
        Y[:, sl] = res.results[cidx]["yt"].T.astype(np.float32)
    return Y
